# revision 10
# baseline (speedup 1.0000x reference)
"""MoE FFN (shared + top-2 routed experts) on 8 Trainium2 NeuronCores.

Strategy (expert-parallel with host-side token dispatch):
  - Router (logits -> top-2 -> softmax) is computed on the host in float64;
    it is a tiny [T,768]@[768,8] matmul. The resulting combine weights and
    per-expert token lists define the dispatch.
  - Core c processes: (a) the tokens routed to expert c (gathered, padded to
    C_pad), through expert c's SwiGLU weights, scaled by combine weight;
    (b) a 512-token slice of all tokens through the shared expert.
  - Each core returns [C_pad + 512, 768] fp32 partial outputs; the host
    scatter-adds routed partials and adds the shared slices.

Device kernel (per core, SPMD — identical program, different data):
  All operands are uploaded in a partition-major pre-shuffled layout so
  each tensor lands in SBUF with a single large contiguous-line DMA
  (128 descriptors each), spread across engines so descriptor
  generation is not serialized. x^T and weights are bf16. mm1 computes
  H^T = (W.T)(X^T) per 128-wide F chunk accumulating over D in PSUM
  (fp32), ACT applies SiLU, DVE multiplies gate*up and casts to bf16.
  mm2 contracts over F with H^T chunks as the stationary operand,
  producing [128 rows, 768] fp32, scaled by the per-row combine weight
  (per-partition tensor_scalar) and DMA'd out.
"""

import numpy as np
import ml_dtypes

import concourse.bass as bass  # noqa: F401
import concourse.mybir as mybir
import concourse.tile as tile
from concourse import bacc
from concourse.bass_utils import run_bass_kernel_spmd

BF16 = ml_dtypes.bfloat16

D = 768
F = 1536
E = 8
K_ACTIVE = 2
NCORES = 8
P = 128
KD = D // P   # 6 contraction chunks for mm1
KF = F // P   # 12 contraction chunks for mm2
NQ = 4        # F-quarters for gate/up weight DMA granularity
FQ = F // NQ  # 384

_prog_cache = {}


def _route_host(xf, w_router, expert_bias):
    """Top-2 routing in float64 (matches the fp32 jax reference to ~1e-7)."""
    T = xf.shape[0]
    logits = xf.astype(np.float64) @ w_router.astype(np.float64)
    logits += expert_bias.astype(np.float64)[None, :]
    ar = np.arange(T)
    i1 = np.argmax(logits, axis=1)
    v1 = logits[ar, i1]
    l2 = logits.copy()
    l2[ar, i1] = -np.inf
    i2 = np.argmax(l2, axis=1)
    v2 = logits[ar, i2]
    e2 = np.exp(v2 - v1)
    s = 1.0 + e2
    w1 = (1.0 / s).astype(np.float32)
    w2 = (e2 / s).astype(np.float32)
    return i1, i2, w1, w2


def _supertiles(c_pad, r_total):
    """Split rows into (start, size, weight_set) chunks of <=512 rows."""
    out = []
    r0 = 0
    while r0 < c_pad:
        ns = min(512, c_pad - r0)
        out.append((r0, ns, 0))
        r0 += ns
    while r0 < r_total:
        ns = min(512, r_total - r0)
        out.append((r0, ns, 1))
        r0 += ns
    return out


def _shuf_gu(w):
    """[D, F] -> [128, NQ*KD*FQ]: partition-major, quarter-major layout."""
    return np.ascontiguousarray(
        w.reshape(KD, P, NQ, FQ).transpose(1, 2, 0, 3).reshape(P, NQ * KD * FQ))


def _shuf_wd(w):
    """[F, D] -> [128, KF*D]: partition-major layout."""
    return np.ascontiguousarray(
        w.reshape(KF, P, D).transpose(1, 0, 2).reshape(P, KF * D))


def _shuf_xt(xt, sts):
    """[D, R] -> [128, KD*R] with per-supertile blocks."""
    out = np.empty((P, KD * xt.shape[1]), xt.dtype)
    for (r0, ns, _) in sts:
        blk = xt[:, r0:r0 + ns].reshape(KD, P, ns).transpose(1, 0, 2)
        out[:, KD * r0:KD * (r0 + ns)] = blk.reshape(P, KD * ns)
    return np.ascontiguousarray(out)


def _build_program(r_total, c_pad):
    dt = mybir.dt
    nc = bacc.Bacc("TRN2", target_bir_lowering=False, debug=False)
    xt_d = nc.dram_tensor("xt", [P, KD * r_total], dt.bfloat16,
                          kind="ExternalInput")
    wg_d = [nc.dram_tensor(f"wg{s}", [P, NQ * KD * FQ], dt.bfloat16,
                           kind="ExternalInput") for s in range(2)]
    wu_d = [nc.dram_tensor(f"wu{s}", [P, NQ * KD * FQ], dt.bfloat16,
                           kind="ExternalInput") for s in range(2)]
    wd_d = [nc.dram_tensor(f"wd{s}", [P, KF * D], dt.bfloat16,
                           kind="ExternalInput") for s in range(2)]
    n_tiles = r_total // P
    sc_d = nc.dram_tensor("scale", [P, n_tiles], dt.float32,
                          kind="ExternalInput")
    out_d = nc.dram_tensor("out", [r_total, D], dt.float32,
                           kind="ExternalOutput")

    silu = mybir.ActivationFunctionType.Silu
    sts = _supertiles(c_pad, r_total)

    with tile.TileContext(nc) as tc:
        with (
            tc.tile_pool(name="const", bufs=1) as const,
            tc.tile_pool(name="work", bufs=3) as work,
            tc.tile_pool(name="outp", bufs=3) as outp,
            tc.tile_pool(name="ps1", bufs=2, space="PSUM") as ps1,
            tc.tile_pool(name="ps2", bufs=2, space="PSUM") as ps2,
        ):
            sc_sb = const.tile([P, n_tiles], dt.float32, tag="sc")
            xt_sb = const.tile([P, KD * r_total], dt.bfloat16, tag="xt")
            wg_sb, wu_sb, wd_sb = [], [], []
            for s in range(2):
                g_t = const.tile([P, NQ, KD, FQ], dt.bfloat16, tag=f"wg{s}")
                u_t = const.tile([P, NQ, KD, FQ], dt.bfloat16, tag=f"wu{s}")
                d_t = const.tile([P, KF, D], dt.bfloat16, tag=f"wd{s}")
                wg_sb.append(g_t)
                wu_sb.append(u_t)
                wd_sb.append(d_t)

            # DMA plan: few large contiguous-line transfers, spread across
            # engines (each dma_start costs ~0.6us of descriptor generation
            # on its issuing engine). Issue order tracks first use.
            # DMA plan: weights arrive quarter-by-quarter in the order the
            # f-major mm1 loop consumes them (~90GB/s steady demand), so
            # only x^T + the first quarter gates the first matmul.
            r0, ns0, _ = sts[0]
            nc.sync.dma_start(out=sc_sb, in_=sc_d[:, :])
            nc.sync.dma_start(out=xt_sb[:, :KD * ns0], in_=xt_d[:, :KD * ns0])
            eng = [nc.gpsimd, nc.scalar]
            for q in range(NQ):
                qs = slice(q * KD * FQ, (q + 1) * KD * FQ)
                for i, (w_sb, w_d) in enumerate(
                        [(wg_sb, wg_d), (wu_sb, wu_d)]):
                    for s in range(2):
                        eng[(2 * i + s) % 2].dma_start(
                            out=w_sb[s][:, q],
                            in_=w_d[s][:, qs].rearrange("p (k f) -> p k f", k=KD))
                if q == 0:
                    for (rr, nn, _s) in sts[1:]:
                        nc.sync.dma_start(
                            out=xt_sb[:, KD * rr:KD * (rr + nn)],
                            in_=xt_d[:, KD * rr:KD * (rr + nn)])
            nc.gpsimd.dma_start(out=wd_sb[0][:], in_=wd_d[0][:, :]
                                .rearrange("p (k d) -> p k d", k=KF))
            nc.scalar.dma_start(out=wd_sb[1][:], in_=wd_d[1][:, :]
                                .rearrange("p (k d) -> p k d", k=KF))

            # Phase 1 (f-major): H^T[f] for all rows, both weight sets.
            h = const.tile([P, KF, r_total], dt.bfloat16, tag="h")
            for f in range(KF):
                q, fr = divmod(f, KF // NQ)
                fs = slice(fr * P, (fr + 1) * P)
                for (r0, ns, s) in sts:
                    pg = ps1.tile([P, ns], dt.float32, tag="pg")
                    pu = ps1.tile([P, ns], dt.float32, tag="pu")
                    for k in range(KD):
                        rhs = xt_sb[:, KD * r0 + k * ns: KD * r0 + (k + 1) * ns]
                        nc.tensor.matmul(pg, wg_sb[s][:, q, k, fs], rhs,
                                         start=(k == 0), stop=(k == KD - 1))
                    for k in range(KD):
                        rhs = xt_sb[:, KD * r0 + k * ns: KD * r0 + (k + 1) * ns]
                        nc.tensor.matmul(pu, wu_sb[s][:, q, k, fs], rhs,
                                         start=(k == 0), stop=(k == KD - 1))
                    sg = work.tile([P, ns], dt.float32, tag="sg")
                    nc.scalar.activation(sg, pg, silu)
                    nc.vector.tensor_mul(h[:, f, r0:r0 + ns], sg, pu)

            # Phase 2: down-projection per 128-row tile, combine-scale, store.
            for (r0, ns, s) in sts:
                for sub in range(ns // P):
                    rt = r0 // P + sub
                    po0 = ps2.tile([P, 384], dt.float32, tag="po0")
                    po1 = ps2.tile([P, 384], dt.float32, tag="po1")
                    ss = slice(rt * P, (rt + 1) * P)
                    for f in range(KF):
                        nc.tensor.matmul(po0, h[:, f, ss], wd_sb[s][:, f, 0:384],
                                         start=(f == 0), stop=(f == KF - 1))
                        nc.tensor.matmul(po1, h[:, f, ss], wd_sb[s][:, f, 384:768],
                                         start=(f == 0), stop=(f == KF - 1))
                    ob = outp.tile([P, D], dt.float32, tag="ob")
                    nc.vector.tensor_scalar_mul(ob[:, 0:384], po0,
                                                sc_sb[:, rt:rt + 1])
                    nc.vector.tensor_scalar_mul(ob[:, 384:768], po1,
                                                sc_sb[:, rt:rt + 1])
                    nc.sync.dma_start(out=out_d[rt * P:(rt + 1) * P, :], in_=ob)

    nc.compile()
    return nc


def _get_program(r_total, c_pad):
    key = (r_total, c_pad)
    if key not in _prog_cache:
        _prog_cache[key] = _build_program(r_total, c_pad)
    return _prog_cache[key]


def prepare(x, Wg_s, Wu_s, Wd_s, Wg_r, Wu_r, Wd_r, W_router, expert_bias):
    """Host-side routing + sharding. Returns (nc, in_maps, assembly info)."""
    x = np.asarray(x, np.float32)
    B, S, _ = x.shape
    T = B * S
    sh = T // NCORES  # shared tokens per core
    xf = x.reshape(T, D)

    i1, i2, w1, w2 = _route_host(xf, np.asarray(W_router, np.float32),
                                 np.asarray(expert_bias, np.float32))

    tok_idx, tok_w = [], []
    for e in range(E):
        m1 = i1 == e
        m2 = i2 == e
        idx = np.concatenate([np.nonzero(m1)[0], np.nonzero(m2)[0]])
        w = np.concatenate([w1[m1], w2[m2]]).astype(np.float32)
        tok_idx.append(idx)
        tok_w.append(w)
    counts = [len(ix) for ix in tok_idx]
    c_pad = max(P, ((max(counts) + P - 1) // P) * P)
    r_total = c_pad + sh
    n_tiles = r_total // P
    sts = _supertiles(c_pad, r_total)

    xt_full = np.ascontiguousarray(xf.T.astype(BF16))  # [D, T]

    def wcast(a):
        return np.asarray(a, np.float32).astype(BF16)

    wg_s = _shuf_gu(wcast(Wg_s[0]))
    wu_s = _shuf_gu(wcast(Wu_s[0]))
    wd_s = _shuf_wd(wcast(Wd_s[0]))
    in_maps = []
    for c in range(E):
        xt = np.zeros((D, r_total), BF16)
        xt[:, :counts[c]] = xt_full[:, tok_idx[c]]
        xt[:, c_pad:c_pad + sh] = xt_full[:, c * sh:(c + 1) * sh]
        scale = np.zeros(r_total, np.float32)
        scale[:counts[c]] = tok_w[c]
        scale[c_pad:c_pad + sh] = 1.0
        scale_t = np.ascontiguousarray(scale.reshape(n_tiles, P).T)
        in_maps.append({
            "xt": _shuf_xt(xt, sts),
            "wg0": _shuf_gu(wcast(Wg_r[c])),
            "wu0": _shuf_gu(wcast(Wu_r[c])),
            "wd0": _shuf_wd(wcast(Wd_r[c])),
            "wg1": wg_s, "wu1": wu_s, "wd1": wd_s,
            "scale": scale_t,
        })

    nc = _get_program(r_total, c_pad)
    info = dict(T=T, B=B, S=S, sh=sh, c_pad=c_pad, counts=counts,
                tok_idx=tok_idx)
    return nc, in_maps, info


def assemble(results, info):
    T, sh, c_pad = info["T"], info["sh"], info["c_pad"]
    out = np.zeros((T, D), np.float32)
    for c in range(NCORES):
        o = results[c]["out"]
        cnt = info["counts"][c]
        if cnt:
            out[info["tok_idx"][c]] += o[:cnt]
        out[c * sh:(c + 1) * sh] += o[c_pad:c_pad + sh]
    return out.reshape(info["B"], info["S"], D)


def kernel(x, Wg_s, Wu_s, Wd_s, Wg_r, Wu_r, Wd_r, W_router, expert_bias):
    nc, in_maps, info = prepare(x, Wg_s, Wu_s, Wd_s, Wg_r, Wu_r, Wd_r,
                                W_router, expert_bias)
    res = run_bass_kernel_spmd(nc, in_maps, list(range(NCORES)))
    return assemble(res.results, info)


# revision 11
# speedup vs baseline: 1.0467x; 1.0467x over previous
"""MoE FFN (shared + top-2 routed experts) on 8 Trainium2 NeuronCores.

Strategy (expert-parallel with host-side token dispatch):
  - Router (logits -> top-2 -> softmax) is computed on the host in float64;
    it is a tiny [T,768]@[768,8] matmul. The resulting combine weights and
    per-expert token lists define the dispatch.
  - Core c processes: (a) the tokens routed to expert c (gathered, padded to
    C_pad), through expert c's SwiGLU weights, scaled by combine weight;
    (b) a 512-token slice of all tokens through the shared expert.
  - Each core returns [C_pad + 512, 768] fp32 partial outputs; the host
    scatter-adds routed partials and adds the shared slices.

Device kernel (per core, SPMD — identical program, different data):
  All operands are uploaded in a partition-major pre-shuffled layout so
  each tensor lands in SBUF with a single large contiguous-line DMA
  (128 descriptors each), spread across engines so descriptor
  generation is not serialized. x^T and weights are bf16. mm1 computes
  H^T = (W.T)(X^T) per 128-wide F chunk accumulating over D in PSUM
  (fp32), ACT applies SiLU, DVE multiplies gate*up and casts to bf16.
  mm2 contracts over F with H^T chunks as the stationary operand,
  producing [128 rows, 768] fp32, scaled by the per-row combine weight
  (per-partition tensor_scalar) and DMA'd out.
"""

import numpy as np
import ml_dtypes

import concourse.bass as bass  # noqa: F401
import concourse.mybir as mybir
import concourse.tile as tile
from concourse import bacc
from concourse.bass_utils import run_bass_kernel_spmd

BF16 = ml_dtypes.bfloat16

D = 768
F = 1536
E = 8
K_ACTIVE = 2
NCORES = 8
P = 128
KD = D // P   # 6 contraction chunks for mm1
KF = F // P   # 12 contraction chunks for mm2
NQ = 4        # F-quarters for gate/up weight DMA granularity
FQ = F // NQ  # 384

_prog_cache = {}


def _route_host(xf, w_router, expert_bias):
    """Top-2 routing in float64 (matches the fp32 jax reference to ~1e-7)."""
    T = xf.shape[0]
    logits = xf.astype(np.float64) @ w_router.astype(np.float64)
    logits += expert_bias.astype(np.float64)[None, :]
    ar = np.arange(T)
    i1 = np.argmax(logits, axis=1)
    v1 = logits[ar, i1]
    l2 = logits.copy()
    l2[ar, i1] = -np.inf
    i2 = np.argmax(l2, axis=1)
    v2 = logits[ar, i2]
    e2 = np.exp(v2 - v1)
    s = 1.0 + e2
    w1 = (1.0 / s).astype(np.float32)
    w2 = (e2 / s).astype(np.float32)
    return i1, i2, w1, w2


def _supertiles(c_pad, r_total):
    """Split rows into (start, size, weight_set) chunks of <=512 rows."""
    out = []
    r0 = 0
    while r0 < c_pad:
        ns = min(512, c_pad - r0)
        out.append((r0, ns, 0))
        r0 += ns
    while r0 < r_total:
        ns = min(512, r_total - r0)
        out.append((r0, ns, 1))
        r0 += ns
    return out


def _shuf_gu(w):
    """[D, F] -> [128, NQ*KD*FQ]: partition-major, quarter-major layout."""
    return np.ascontiguousarray(
        w.reshape(KD, P, NQ, FQ).transpose(1, 2, 0, 3).reshape(P, NQ * KD * FQ))


def _shuf_wd(w):
    """[F, D] -> [128, KF*D]: partition-major layout."""
    return np.ascontiguousarray(
        w.reshape(KF, P, D).transpose(1, 0, 2).reshape(P, KF * D))


def _shuf_xt(xt, sts):
    """[D, R] -> [128, KD*R] with per-supertile blocks."""
    out = np.empty((P, KD * xt.shape[1]), xt.dtype)
    for (r0, ns, _) in sts:
        blk = xt[:, r0:r0 + ns].reshape(KD, P, ns).transpose(1, 0, 2)
        out[:, KD * r0:KD * (r0 + ns)] = blk.reshape(P, KD * ns)
    return np.ascontiguousarray(out)


def _build_program(r_total, c_pad):
    dt = mybir.dt
    nc = bacc.Bacc("TRN2", target_bir_lowering=False, debug=False)
    xt_d = nc.dram_tensor("xt", [P, KD * r_total], dt.bfloat16,
                          kind="ExternalInput")
    wg_d = [nc.dram_tensor(f"wg{s}", [P, NQ * KD * FQ], dt.bfloat16,
                           kind="ExternalInput") for s in range(2)]
    wu_d = [nc.dram_tensor(f"wu{s}", [P, NQ * KD * FQ], dt.bfloat16,
                           kind="ExternalInput") for s in range(2)]
    wd_d = [nc.dram_tensor(f"wd{s}", [P, KF * D], dt.bfloat16,
                           kind="ExternalInput") for s in range(2)]
    n_tiles = r_total // P
    sc_d = nc.dram_tensor("scale", [P, n_tiles], dt.float32,
                          kind="ExternalInput")
    out_d = nc.dram_tensor("out", [r_total, D], dt.float32,
                           kind="ExternalOutput")

    silu = mybir.ActivationFunctionType.Silu
    sts = _supertiles(c_pad, r_total)

    with tile.TileContext(nc) as tc:
        with (
            tc.tile_pool(name="const", bufs=1) as const,
            tc.tile_pool(name="work", bufs=3) as work,
            tc.tile_pool(name="outp", bufs=3) as outp,
            tc.tile_pool(name="ps1", bufs=2, space="PSUM") as ps1,
            tc.tile_pool(name="ps2", bufs=2, space="PSUM") as ps2,
        ):
            sc_sb = const.tile([P, n_tiles], dt.float32, tag="sc")
            xt_sb = const.tile([P, KD * r_total], dt.bfloat16, tag="xt")
            wg_sb, wu_sb, wd_sb = [], [], []
            for s in range(2):
                g_t = const.tile([P, NQ, KD, FQ], dt.bfloat16, tag=f"wg{s}")
                u_t = const.tile([P, NQ, KD, FQ], dt.bfloat16, tag=f"wu{s}")
                d_t = const.tile([P, KF, D], dt.bfloat16, tag=f"wd{s}")
                wg_sb.append(g_t)
                wu_sb.append(u_t)
                wd_sb.append(d_t)

            # DMA plan: few large contiguous-line transfers, spread across
            # engines (each dma_start costs ~0.6us of descriptor generation
            # on its issuing engine). Issue order tracks first use.
            # DMA plan, ordered by first-use time in the supertile-major loop
            # (aggregate HBM BW at kernel start is the binding constraint):
            # supertile 0's x^T + the first weight quarter gate the first
            # matmul; subsequent quarters are consumed one per ~7.7us.
            r0, ns0, _ = sts[0]
            nc.sync.dma_start(out=sc_sb, in_=sc_d[:, :])
            nc.sync.dma_start(out=xt_sb[:, :KD * ns0], in_=xt_d[:, :KD * ns0])
            for q in range(NQ):
                qs = slice(q * KD * FQ, (q + 1) * KD * FQ)
                nc.gpsimd.dma_start(
                    out=wg_sb[0][:, q],
                    in_=wg_d[0][:, qs].rearrange("p (k f) -> p k f", k=KD))
                nc.scalar.dma_start(
                    out=wu_sb[0][:, q],
                    in_=wu_d[0][:, qs].rearrange("p (k f) -> p k f", k=KD))
                if q == 1:
                    rr, nn, _s = sts[1]
                    nc.sync.dma_start(out=xt_sb[:, KD * rr:KD * (rr + nn)],
                                      in_=xt_d[:, KD * rr:KD * (rr + nn)])
            nc.gpsimd.dma_start(out=wd_sb[0][:], in_=wd_d[0][:, :]
                                .rearrange("p (k d) -> p k d", k=KF))
            for (rr, nn, _s) in sts[2:]:
                nc.sync.dma_start(out=xt_sb[:, KD * rr:KD * (rr + nn)],
                                  in_=xt_d[:, KD * rr:KD * (rr + nn)])
            nc.scalar.dma_start(out=wg_sb[1][:], in_=wg_d[1][:, :]
                                .rearrange("p (q k f) -> p q k f", q=NQ, k=KD))
            nc.gpsimd.dma_start(out=wu_sb[1][:], in_=wu_d[1][:, :]
                                .rearrange("p (q k f) -> p q k f", q=NQ, k=KD))
            nc.scalar.dma_start(out=wd_sb[1][:], in_=wd_d[1][:, :]
                                .rearrange("p (k d) -> p k d", k=KF))

            for (r0, ns, s) in sts:
                h = work.tile([P, KF, ns], dt.bfloat16, tag="h")
                for f in range(KF):
                    pg = ps1.tile([P, ns], dt.float32, tag="pg")
                    pu = ps1.tile([P, ns], dt.float32, tag="pu")
                    q, fr = divmod(f, KF // NQ)
                    fs = slice(fr * P, (fr + 1) * P)
                    for k in range(KD):
                        rhs = xt_sb[:, KD * r0 + k * ns: KD * r0 + (k + 1) * ns]
                        nc.tensor.matmul(pg, wg_sb[s][:, q, k, fs], rhs,
                                         start=(k == 0), stop=(k == KD - 1))
                    for k in range(KD):
                        rhs = xt_sb[:, KD * r0 + k * ns: KD * r0 + (k + 1) * ns]
                        nc.tensor.matmul(pu, wu_sb[s][:, q, k, fs], rhs,
                                         start=(k == 0), stop=(k == KD - 1))
                    sg = work.tile([P, ns], dt.float32, tag="sg")
                    nc.scalar.activation(sg, pg, silu)
                    nc.vector.tensor_mul(h[:, f, :], sg, pu)
                for sub in range(ns // P):
                    rt = r0 // P + sub
                    po0 = ps2.tile([P, 384], dt.float32, tag="po0")
                    po1 = ps2.tile([P, 384], dt.float32, tag="po1")
                    ss = slice(sub * P, (sub + 1) * P)
                    for f in range(KF):
                        nc.tensor.matmul(po0, h[:, f, ss], wd_sb[s][:, f, 0:384],
                                         start=(f == 0), stop=(f == KF - 1))
                        nc.tensor.matmul(po1, h[:, f, ss], wd_sb[s][:, f, 384:768],
                                         start=(f == 0), stop=(f == KF - 1))
                    ob = outp.tile([P, D], dt.float32, tag="ob")
                    nc.vector.tensor_scalar_mul(ob[:, 0:384], po0,
                                                sc_sb[:, rt:rt + 1])
                    nc.vector.tensor_scalar_mul(ob[:, 384:768], po1,
                                                sc_sb[:, rt:rt + 1])
                    nc.sync.dma_start(out=out_d[rt * P:(rt + 1) * P, :], in_=ob)

    nc.compile()
    return nc


def _get_program(r_total, c_pad):
    key = (r_total, c_pad)
    if key not in _prog_cache:
        _prog_cache[key] = _build_program(r_total, c_pad)
    return _prog_cache[key]


def prepare(x, Wg_s, Wu_s, Wd_s, Wg_r, Wu_r, Wd_r, W_router, expert_bias):
    """Host-side routing + sharding. Returns (nc, in_maps, assembly info)."""
    x = np.asarray(x, np.float32)
    B, S, _ = x.shape
    T = B * S
    sh = T // NCORES  # shared tokens per core
    xf = x.reshape(T, D)

    i1, i2, w1, w2 = _route_host(xf, np.asarray(W_router, np.float32),
                                 np.asarray(expert_bias, np.float32))

    tok_idx, tok_w = [], []
    for e in range(E):
        m1 = i1 == e
        m2 = i2 == e
        idx = np.concatenate([np.nonzero(m1)[0], np.nonzero(m2)[0]])
        w = np.concatenate([w1[m1], w2[m2]]).astype(np.float32)
        tok_idx.append(idx)
        tok_w.append(w)
    counts = [len(ix) for ix in tok_idx]
    c_pad = max(P, ((max(counts) + P - 1) // P) * P)
    r_total = c_pad + sh
    n_tiles = r_total // P
    sts = _supertiles(c_pad, r_total)

    xt_full = np.ascontiguousarray(xf.T.astype(BF16))  # [D, T]

    def wcast(a):
        return np.asarray(a, np.float32).astype(BF16)

    wg_s = _shuf_gu(wcast(Wg_s[0]))
    wu_s = _shuf_gu(wcast(Wu_s[0]))
    wd_s = _shuf_wd(wcast(Wd_s[0]))
    in_maps = []
    for c in range(E):
        xt = np.zeros((D, r_total), BF16)
        xt[:, :counts[c]] = xt_full[:, tok_idx[c]]
        xt[:, c_pad:c_pad + sh] = xt_full[:, c * sh:(c + 1) * sh]
        scale = np.zeros(r_total, np.float32)
        scale[:counts[c]] = tok_w[c]
        scale[c_pad:c_pad + sh] = 1.0
        scale_t = np.ascontiguousarray(scale.reshape(n_tiles, P).T)
        in_maps.append({
            "xt": _shuf_xt(xt, sts),
            "wg0": _shuf_gu(wcast(Wg_r[c])),
            "wu0": _shuf_gu(wcast(Wu_r[c])),
            "wd0": _shuf_wd(wcast(Wd_r[c])),
            "wg1": wg_s, "wu1": wu_s, "wd1": wd_s,
            "scale": scale_t,
        })

    nc = _get_program(r_total, c_pad)
    info = dict(T=T, B=B, S=S, sh=sh, c_pad=c_pad, counts=counts,
                tok_idx=tok_idx)
    return nc, in_maps, info


def assemble(results, info):
    T, sh, c_pad = info["T"], info["sh"], info["c_pad"]
    out = np.zeros((T, D), np.float32)
    for c in range(NCORES):
        o = results[c]["out"]
        cnt = info["counts"][c]
        if cnt:
            out[info["tok_idx"][c]] += o[:cnt]
        out[c * sh:(c + 1) * sh] += o[c_pad:c_pad + sh]
    return out.reshape(info["B"], info["S"], D)


def kernel(x, Wg_s, Wu_s, Wd_s, Wg_r, Wu_r, Wd_r, W_router, expert_bias):
    nc, in_maps, info = prepare(x, Wg_s, Wu_s, Wd_s, Wg_r, Wu_r, Wd_r,
                                W_router, expert_bias)
    res = run_bass_kernel_spmd(nc, in_maps, list(range(NCORES)))
    return assemble(res.results, info)
